# revision 8
# baseline (speedup 1.0000x reference)
"""Causal single-head attention (B=4, T=4096, E=1024, H=64) on 8 trn2 cores.

Sharding: core = b*2 + kh. Each core handles batch b and the key-parity
shard kh (256-row key blocks with global index g, g%2==kh). Scores use the
identity exp(s) without max-subtraction (s ~ N(0,1)), so partial outputs
combine across the key shards by plain addition of the unnormalized
numerator and the row-sum (carried as a 65th "ones" column of V).

To keep one SPMD program for all cores, the host feeds each core x[b].T
with 256-column blocks pairwise swapped for kh=1, so "own-parity" key
blocks always sit at even local positions. The causal window for local
q-block l is then always local k-tiles [0, 2*(l//2+1)), with a data-driven
mask (DMA'd per core) on the last two k-tiles. Output is the unnormalized
[65, 4096] transposed accumulator; the host un-permutes, sums the two
shards and divides by the row-sum.
"""
import sys
import numpy as np

sys.path.insert(0, "/opt/trn_rl_repo")

import concourse.bass as bass
import concourse.bacc as bacc
import concourse.tile as tile
from concourse import mybir
from concourse.bass_utils import run_bass_kernel_spmd

B, T, E, H = 4, 4096, 1024, 64
P = 128
QB = 256                  # q/k block granularity
NB = T // QB              # 16 local q-blocks
NE = E // P               # 8 contraction chunks
NCB = NB // 2             # 8 projection column blocks of 512
ST_CHUNK = 4              # k-tiles per score/exp chunk ([128, 1024] psum)
F32 = mybir.dt.float32
F32R = mybir.dt.float32r


def build_nc(mm_dt=F32R, loop_n=None, debug=False):
    nc = bacc.Bacc()
    xt_d = nc.declare_dram_parameter("xt", [E, T], mm_dt, isOutput=False)
    wq_d = nc.declare_dram_parameter("wq", [E, H], mm_dt, isOutput=False)
    wkv_d = nc.declare_dram_parameter("wkv", [E, 2 * H], mm_dt, isOutput=False)
    id_d = nc.declare_dram_parameter("ident", [H, H], F32, isOutput=False)
    mask_d = nc.declare_dram_parameter("mask", [P, 2, 2 * QB], mm_dt, isOutput=False)
    o_d = nc.declare_dram_parameter("o_un", [H + 1, T], F32, isOutput=True)
    if debug:
        qt_dbg = nc.declare_dram_parameter("qt_dbg", [H, T], F32, isOutput=True)
        kt_dbg = nc.declare_dram_parameter("kt_dbg", [H, T // 2], F32, isOutput=True)
        v_dbg = nc.declare_dram_parameter("v_dbg", [T // 2, H + 1], F32, isOutput=True)
        pt_dbg = nc.declare_dram_parameter("pt_dbg", [P, 2 * QB], F32, isOutput=True)

    with tile.TileContext(nc) as tc:
        with tc.tile_pool(name="const", bufs=1) as const, \
             tc.tile_pool(name="persist", bufs=1) as persist, \
             tc.tile_pool(name="xtp", bufs=4) as xtp, \
             tc.tile_pool(name="ptp", bufs=3) as ptp, \
             tc.tile_pool(name="outp", bufs=4) as outp, \
             tc.tile_pool(name="acc", bufs=2, space="PSUM") as accp, \
             tc.tile_pool(name="stp", bufs=2, space="PSUM") as stp:

            wq_t, wkv_t = [], []
            for e in range(NE):
                tq = const.tile([P, H], mm_dt, tag=f"wq{e}", name=f"wq{e}")
                nc.sync.dma_start(out=tq, in_=wq_d[e * P:(e + 1) * P, :])
                wq_t.append(tq)
                tk = const.tile([P, 2 * H], mm_dt, tag=f"wkv{e}", name=f"wkv{e}")
                nc.sync.dma_start(out=tk, in_=wkv_d[e * P:(e + 1) * P, :])
                wkv_t.append(tk)
            id_sb = const.tile([H, H], F32, tag="ident", name="id_sb")
            nc.sync.dma_start(out=id_sb, in_=id_d[:, :])
            m_sb = const.tile([P, 2, 2 * QB], mm_dt, tag="mask", name="m_sb")
            nc.sync.dma_start(out=m_sb, in_=mask_d[:, :, :])

            ones_sb = const.tile([P, 1], F32, tag="ones", name="ones_sb")
            nc.vector.memset(ones_sb, 1.0)

            qt_tiles = [persist.tile([H, QB], mm_dt, tag=f"qt{i}", name=f"qt{i}") for i in range(NB)]
            kt_tiles = [persist.tile([H, QB], mm_dt, tag=f"kt{i}", name=f"kt{i}") for i in range(NCB)]
            vtt_tiles = [persist.tile([H, QB], F32, tag=f"vtt{i}", name=f"vtt{i}") for i in range(NCB)]
            v_tiles = [persist.tile([P, H + 1], mm_dt, tag=f"v{i}", name=f"v{i}") for i in range(2 * NCB)]

            pt_hold = []

            def body(_iv=None):
                # ---- Phase A: projections (QT full, KT/V own-parity) ----
                for cb in range(NCB):
                    acc = accp.tile([P, 768], F32, tag="acc", name="accA")
                    qt_ps = acc[0:H, 0:512]
                    kvt_ps = acc[:, 512:768]
                    for e in range(NE):
                        xt_t = xtp.tile([P, 512], mm_dt, tag="xt", name="xt_t")
                        nc.sync.dma_start(
                            out=xt_t,
                            in_=xt_d[e * P:(e + 1) * P, cb * 512:(cb + 1) * 512])
                        st, sp = (e == 0), (e == NE - 1)
                        nc.tensor.matmul(qt_ps, lhsT=wq_t[e], rhs=xt_t,
                                         start=st, stop=sp)
                        nc.tensor.matmul(kvt_ps, lhsT=wkv_t[e], rhs=xt_t[:, 0:QB],
                                         start=st, stop=sp)
                    for s in range(2):
                        nc.vector.tensor_copy(qt_tiles[2 * cb + s],
                                              qt_ps[:, s * QB:(s + 1) * QB])
                    nc.vector.tensor_copy(kt_tiles[cb], kvt_ps[0:H, :])
                    nc.vector.tensor_copy(vtt_tiles[cb], kvt_ps[H:2 * H, :])
                    # V natural: PE-transpose the two 128-col k-tiles of VT
                    for j in range(2):
                        i = 2 * cb + j
                        vtp = stp.tile([P, ST_CHUNK * QB], F32, tag="st",
                                       name="vtp")[:, 0:H]
                        nc.tensor.transpose(
                            vtp, vtt_tiles[cb][:, j * P:(j + 1) * P], id_sb)
                        nc.vector.tensor_copy(v_tiles[i][:, 0:H], vtp)
                        nc.vector.tensor_copy(v_tiles[i][:, H:H + 1], ones_sb)

                # ---- Phase C: attention ----
                for l in range(NB):
                    nkt = 2 * (l // 2 + 1)     # k-tiles in causal window
                    ot = accp.tile([P, 768], F32, tag="acc", name="accC")[0:H + 1, 0:QB]
                    for cs in range(0, nkt, ST_CHUNK):
                        n = min(ST_CHUNK, nkt - cs)
                        st_ps = stp.tile([P, ST_CHUNK * QB], F32, tag="st", name="st_ps")
                        for idx in range(n):
                            t = cs + idx
                            nc.tensor.matmul(
                                st_ps[:, idx * QB:(idx + 1) * QB],
                                lhsT=kt_tiles[t // 2][:, (t % 2) * P:(t % 2 + 1) * P],
                                rhs=qt_tiles[l],
                                start=True, stop=True)
                        pt = ptp.tile([P, ST_CHUNK * QB], mm_dt, tag="pt", name="pt")
                        if debug and l == 1 and cs == 0:
                            pt_hold.append(pt)
                        nc.scalar.activation(pt[:, 0:n * QB], st_ps[:, 0:n * QB],
                                             mybir.ActivationFunctionType.Exp,
                                             scale=0.125)
                        for j in range(2):
                            t = nkt - 2 + j
                            if cs <= t < cs + n:
                                sl = pt[:, (t - cs) * QB:(t - cs + 1) * QB]
                                nc.vector.tensor_mul(
                                    sl, sl, m_sb[:, l % 2, j * QB:(j + 1) * QB])
                        for idx in range(n):
                            t = cs + idx
                            nc.tensor.matmul(
                                ot, lhsT=v_tiles[t],
                                rhs=pt[:, idx * QB:(idx + 1) * QB],
                                start=(t == 0), stop=(t == nkt - 1))
                    o_t = outp.tile([H + 1, QB], F32, tag="o", name="o_t")
                    nc.vector.tensor_copy(o_t, ot)
                    nc.sync.dma_start(out=o_d[:, l * QB:(l + 1) * QB], in_=o_t[:, :])
                    if debug and l == 1:
                        dscr = outp.tile([P, 2 * QB], F32, tag="dscr", name="dscr")
                        nc.vector.tensor_copy(dscr, pt_hold[0][:, 0:2 * QB].bitcast(F32))
                        nc.sync.dma_start(out=pt_dbg[:, :], in_=dscr[:, :])

                if debug:
                    for i in range(NB):
                        nc.sync.dma_start(out=qt_dbg[:, i * QB:(i + 1) * QB],
                                          in_=qt_tiles[i][:, :].bitcast(F32))
                    for i in range(NCB):
                        nc.sync.dma_start(out=kt_dbg[:, i * QB:(i + 1) * QB],
                                          in_=kt_tiles[i][:, :].bitcast(F32))
                    for i in range(2 * NCB):
                        nc.sync.dma_start(out=v_dbg[i * P:(i + 1) * P, :],
                                          in_=v_tiles[i][:, :].bitcast(F32))

            if loop_n is not None and loop_n > 1:
                with tc.For_i(0, loop_n, 1):
                    body()
            else:
                body()

    nc.compile()
    return nc


def _host_prep(x, Wq, Wk, Wv):
    in_maps = []
    kq = np.arange(P)[:, None]
    qq = np.arange(QB)[None, :]
    for b in range(B):
        for kh in range(2):
            xt = np.ascontiguousarray(x[b].T)
            if kh == 1:
                xt = np.ascontiguousarray(
                    xt.reshape(E, NB // 2, 2, QB)[:, :, ::-1, :].reshape(E, T))
            mask = np.zeros((P, 2, 2 * QB), np.float32)
            for j in range(2):
                mask[:, 0, j * QB:(j + 1) * QB] = ((j * P + kq) <= qq)
            mask[:, 1, :] = 1.0 if kh == 0 else 0.0
            in_maps.append({"xt": xt, "wq": np.ascontiguousarray(Wq),
                            "wkv": np.ascontiguousarray(
                                np.concatenate([Wk, Wv], axis=1)),
                            "ident": np.eye(H, dtype=np.float32), "mask": mask})
    return in_maps


def _host_combine(results):
    out = np.zeros((B, T, H), np.float32)
    for b in range(B):
        o0 = results[2 * b]["o_un"]
        o1 = results[2 * b + 1]["o_un"]
        o1 = o1.reshape(H + 1, NB // 2, 2, QB)[:, :, ::-1, :].reshape(H + 1, T)
        s = o0 + o1
        out[b] = (s[:H] / s[H:H + 1]).T
    return out


_NC_CACHE = {}


def kernel(x, Wq, Wk, Wv):
    x = np.asarray(x, np.float32)
    key = "main"
    if key not in _NC_CACHE:
        _NC_CACHE[key] = build_nc()
    nc = _NC_CACHE[key]
    in_maps = _host_prep(x, np.asarray(Wq, np.float32),
                         np.asarray(Wk, np.float32), np.asarray(Wv, np.float32))
    res = run_bass_kernel_spmd(nc, in_maps, core_ids=list(range(8)))
    return _host_combine(res.results)
